# revision 1
# baseline (speedup 1.0000x reference)
"""Trainium2 Bass kernel for DiversityInjection (MoE-style per-agent low-rank
perturbation + LayerNorm).

Strategy: expert-parallel over the 256 agents. The host routes tokens to the
core that owns their agent (MoE dispatch done host-side), packs them into
fixed-capacity per-agent slots (CAP tokens), and each core runs dense batched
matmuls over groups of G=3 slots (126 tokens per group tile):

  mm1 (3 slots at once): psum1[96, 126] = [U_a|U_b|U_c]^T @ hT3
        8 contract chunks of 128; useful output = 3 diagonal [32, 42] blocks
  mm2 (3 slots at once, block-diag): psum2[126, 512] =
        blockdiag(intT_a, intT_b, intT_c)^T(96x126) @ [V_a; V_b; V_c](96x512)
  out = LayerNorm(h + pert) fused via bn_stats + scalar activation

The padded output is scattered back to original token order on the host.
"""

import os
import sys

for _p in ("/opt/trn_rl_repo", "/root/.axon_site/_ro/trn_rl_repo"):
    if os.path.isdir(_p) and _p not in sys.path:
        sys.path.insert(0, _p)

import numpy as np

N_CORES = 8
CAP = 42           # tokens per slot (per-agent capacity)
G = 3              # slots per group tile (G*CAP <= 128, G*rank <= 128)
ALPHA_MAX = 5.0
LN_EPS = 1e-5
VARIANT = os.environ.get("BASS_KERNEL_VARIANT", "bf16")

_PROGRAM_CACHE = {}


def _reference_numpy(h, log_alpha, ln_gamma, ln_beta, projection_u, projection_v,
                     agent_ids):
    """Fallback pure-numpy implementation (used only if packing does not fit)."""
    num_agents = projection_u.shape[0]
    ids = agent_ids % num_agents
    alpha = min(np.exp(np.float32(log_alpha)), np.float32(ALPHA_MAX))
    out = np.empty_like(h)
    for a in range(num_agents):
        m = ids == a
        if not m.any():
            continue
        hb = h[m]
        pert = (hb @ projection_u[a]) @ projection_v[a]
        out[m] = hb + alpha * pert
    mean = out.mean(-1, keepdims=True, dtype=np.float64)
    var = out.var(-1, keepdims=True, dtype=np.float64)
    out = (out - mean) / np.sqrt(var + LN_EPS)
    return (out * ln_gamma + ln_beta).astype(h.dtype)


def _build_program(nslot, hidden, rank, variant):
    """Build the per-core Bass program. Same program runs SPMD on all 8 cores."""
    from contextlib import ExitStack

    import concourse.bacc as bacc
    import concourse.mybir as mybir
    import concourse.tile as tile

    assert hidden == 1024 and rank == 32
    assert nslot % G == 0
    ngroup = nslot // G
    nchunk = hidden // 128
    T = G * CAP          # tokens per group tile (126)
    KR = G * rank        # stacked rank (96)

    if variant == "bf16":
        mmdt = mybir.dt.bfloat16
    elif variant.endswith("r"):
        mmdt = mybir.dt.float32r
    else:
        mmdt = mybir.dt.float32

    nc = bacc.Bacc("TRN2", target_bir_lowering=False, debug=False)

    u_d = nc.dram_tensor("u_sw", (ngroup, 128, nchunk * KR), mmdt,
                         kind="ExternalInput")
    v_d = nc.dram_tensor("v_sw", (ngroup, KR, hidden), mmdt,
                         kind="ExternalInput")
    hT_d = nc.dram_tensor("hT_sw", (ngroup, 128, nchunk * T), mmdt,
                          kind="ExternalInput")
    id_d = nc.dram_tensor("ident", (128, 128), mmdt, kind="ExternalInput")
    outdt = mybir.dt.bfloat16 if variant == "bf16" else mybir.dt.float32
    out_d = nc.dram_tensor("out_pk", (ngroup, T, hidden), outdt,
                           kind="ExternalOutput")

    with tile.TileContext(nc) as tc, ExitStack() as ctx:
        upool = ctx.enter_context(tc.tile_pool(name="u", bufs=6))
        vpool = ctx.enter_context(tc.tile_pool(name="v", bufs=6))
        htpool = ctx.enter_context(tc.tile_pool(name="hT", bufs=6))
        bpool = ctx.enter_context(tc.tile_pool(name="blk", bufs=3))
        spool = ctx.enter_context(tc.tile_pool(name="stats", bufs=8))
        opool = ctx.enter_context(tc.tile_pool(name="o", bufs=6))
        cpool = ctx.enter_context(tc.tile_pool(name="const", bufs=1))
        p1pool = ctx.enter_context(tc.tile_pool(name="psum1", bufs=2, space="PSUM"))
        p2pool = ctx.enter_context(tc.tile_pool(name="psum2", bufs=3, space="PSUM"))

        eps_t = cpool.tile([128, 1], mybir.dt.float32)
        nc.vector.memset(eps_t[:], LN_EPS)
        id_t = cpool.tile([128, 128], mmdt)
        nc.gpsimd.dma_start(id_t[:], id_d[:])

        # persistent blk buffers: off-diagonal zeros are written once and
        # survive across groups (only the diagonal blocks are rewritten)
        NBLK = 3
        blk_bufs = [cpool.tile([KR, T], mmdt, name=f"blk{i}")
                    for i in range(NBLK)]
        for b in blk_bufs:
            nc.gpsimd.memset(b[:], 0.0)

        def issue_dmas(g):
            u_t = upool.tile([128, nchunk * KR], mmdt)
            nc.scalar.dma_start(u_t[:], u_d[g])
            hT_t = htpool.tile([128, nchunk * T], mmdt)
            nc.sync.dma_start(hT_t[:], hT_d[g])
            v_t = vpool.tile([KR, hidden], mmdt)
            nc.sync.dma_start(v_t[:], v_d[g])
            return u_t, hT_t, v_t

        def emit_mm1(u_t, hT_t):
            psum1 = p1pool.tile([KR, T], mybir.dt.float32)
            for c in range(nchunk):
                nc.tensor.matmul(
                    psum1[:],
                    u_t[:, c * KR:(c + 1) * KR],
                    hT_t[:, c * T:(c + 1) * T],
                    start=(c == 0), stop=(c == nchunk - 1),
                )
            return psum1

        def emit_casts(g, psum1):
            # block-diag [KR, T] lhsT: diag [rank, CAP] blocks from psum1
            blk = blk_bufs[g % NBLK]
            for s in range(G):
                nc.vector.tensor_copy(
                    blk[s * rank:(s + 1) * rank,
                        s * CAP:(s + 1) * CAP],
                    psum1[s * rank:(s + 1) * rank,
                          s * CAP:(s + 1) * CAP])
            return blk

        # software-pipelined: group g's mm2/LN overlaps group g+1's mm1 and
        # blk casts, so the PE never stalls behind the DVE LayerNorm chain
        dmas = [issue_dmas(g) for g in range(min(3, ngroup))]
        p1 = emit_mm1(dmas[0][0], dmas[0][1])
        cur = (emit_casts(0, p1), dmas[0][1], dmas[0][2])

        for g in range(ngroup):
            blk, hT_t, v_t = cur
            if g + 3 < ngroup:
                dmas.append(issue_dmas(g + 3))

            # per 512-wide half: mm2 for the half, then the residual-h
            # transposes for that half (regular matmul vs identity so bf16
            # ins accumulate into fp32 psum), then bn_stats on it — LN
            # stats of half 0 overlap the PE filling half 1
            psum2 = p2pool.tile([128, hidden], mybir.dt.float32)
            stats = spool.tile([128, 6 * (hidden // 512)], mybir.dt.float32)

            def emit_half(q):
                nc.tensor.matmul(
                    psum2[0:T, q * 512:(q + 1) * 512],
                    blk[:],
                    v_t[:, q * 512:(q + 1) * 512],
                    start=True, stop=True,
                )
                for c in range(4 * q, 4 * (q + 1)):
                    nc.tensor.matmul(
                        psum2[0:T, c * 128:(c + 1) * 128],
                        hT_t[:, c * T:(c + 1) * T],
                        id_t[:],
                        start=False, stop=True,
                        skip_group_check=True,
                    )
                nc.vector.bn_stats(stats[0:T, q * 6:(q + 1) * 6],
                                   psum2[0:T, q * 512:(q + 1) * 512])

            emit_half(0)
            if g + 1 < ngroup:
                nxt = dmas[g + 1]
                p1 = emit_mm1(nxt[0], nxt[1])
                cur = (emit_casts(g + 1, p1), nxt[1], nxt[2])
            emit_half(1)

            aggr = spool.tile([128, 2], mybir.dt.float32)
            nc.vector.bn_aggr(aggr[0:T, :],
                              stats[0:T, :].rearrange("p (c s) -> p c s", s=3))
            std = spool.tile([128, 1], mybir.dt.float32)
            nc.scalar.activation(std[0:T, :], aggr[0:T, 1:2],
                                 mybir.ActivationFunctionType.Sqrt,
                                 bias=eps_t[0:T, 0:1])
            rstd = spool.tile([128, 1], mybir.dt.float32)
            nc.vector.reciprocal(rstd[0:T, :], std[0:T, :])
            nmr = spool.tile([128, 1], mybir.dt.float32)
            nc.vector.scalar_tensor_tensor(nmr[0:T, :], aggr[0:T, 0:1], -1.0,
                                           rstd[0:T, :],
                                           mybir.AluOpType.mult,
                                           mybir.AluOpType.mult)
            o_t = opool.tile([128, hidden], outdt)
            nc.scalar.activation(o_t[0:T, :], psum2[0:T, :],
                                 mybir.ActivationFunctionType.Identity,
                                 bias=nmr[0:T, 0:1], scale=rstd[0:T, 0:1])
            if g % 2 == 0:
                nc.gpsimd.dma_start(out_d[g], o_t[0:T, :])
            else:
                nc.sync.dma_start(out_d[g], o_t[0:T, :])

    nc.finalize()
    return nc


def _get_program(nslot, hidden, rank, variant):
    key = (nslot, hidden, rank, variant)
    if key not in _PROGRAM_CACHE:
        _PROGRAM_CACHE[key] = _build_program(nslot, hidden, rank, variant)
    return _PROGRAM_CACHE[key]


def kernel(h, log_alpha, ln_gamma, ln_beta, projection_u, projection_v,
           agent_ids):
    h = np.asarray(h, dtype=np.float32)
    projection_u = np.asarray(projection_u, dtype=np.float32)
    projection_v = np.asarray(projection_v, dtype=np.float32)
    ln_gamma = np.asarray(ln_gamma, dtype=np.float32)
    ln_beta = np.asarray(ln_beta, dtype=np.float32)
    ids_raw = np.asarray(agent_ids)
    log_alpha = np.float32(np.asarray(log_alpha))

    B, H = h.shape
    A, H2, R = projection_u.shape
    ids = (ids_raw.astype(np.int64) % A).astype(np.int32)

    if H != 1024 or H2 != H or R != 32 or projection_v.shape != (A, R, H):
        return _reference_numpy(h, log_alpha, ln_gamma, ln_beta, projection_u,
                                projection_v, agent_ids)

    alpha = np.float32(min(np.exp(log_alpha), np.float32(ALPHA_MAX)))
    use_gamma = not np.all(ln_gamma == 1.0)
    use_beta = not np.all(ln_beta == 0.0)

    # ---- host-side MoE dispatch: sort tokens by agent, build capacity slots
    order = np.argsort(ids, kind="stable").astype(np.int64)
    counts = np.bincount(ids, minlength=A)
    starts = np.zeros(A + 1, np.int64)
    np.cumsum(counts, out=starts[1:])

    slot_agent = []   # agent id per slot
    slot_rows = []    # (start, n) into `order` per slot
    for a in range(A):
        n = int(counts[a])
        s = int(starts[a])
        while n > 0:
            take = min(n, CAP)
            slot_agent.append(a)
            slot_rows.append((s, take))
            s += take
            n -= take
    total_slots = len(slot_agent)
    nslot = -(-total_slots // N_CORES)
    nslot = max(nslot, G)
    if nslot % G:
        nslot += G - nslot % G
    if nslot > 96:  # way off the expected distribution; play it safe
        return _reference_numpy(h, log_alpha, ln_gamma, ln_beta, projection_u,
                                projection_v, agent_ids)
    while len(slot_agent) < nslot * N_CORES:
        slot_agent.append(0)
        slot_rows.append((0, 0))
    slot_agent = np.asarray(slot_agent, np.int64)

    ngroup = nslot // G
    nchunk = H // 128
    T = G * CAP
    KR = G * R

    # row_idx: global token index feeding each padded row (clamped for pads)
    nrows = nslot * CAP
    row_idx = np.zeros((N_CORES, nrows), np.int64)
    row_valid = np.zeros((N_CORES, nrows), bool)
    for j, (s, n) in enumerate(slot_rows):
        core, sl = divmod(j, nslot)
        r0 = sl * CAP
        if n:
            row_idx[core, r0:r0 + n] = order[s:s + n]
            row_valid[core, r0:r0 + n] = True

    if VARIANT == "bf16":
        import ml_dtypes
        mmdt_np = ml_dtypes.bfloat16
    else:
        mmdt_np = np.float32

    h_pk = h[row_idx].astype(mmdt_np).reshape(N_CORES, ngroup, T, H)
    # hT per group: [p(128), c(8), t(T)]
    hT_sw = np.ascontiguousarray(
        h_pk.reshape(N_CORES, ngroup, T, nchunk, 128)
        .transpose(0, 1, 4, 3, 2)).reshape(N_CORES, ngroup, 128, nchunk * T)
    ident = np.eye(128, dtype=mmdt_np)

    sa = slot_agent.reshape(N_CORES, nslot)
    # u: [g, p(128), c(8), s(G), r(32)]
    u_sw = np.ascontiguousarray(
        projection_u[sa].astype(mmdt_np)                  # [8, ns, H, R]
        .reshape(N_CORES, ngroup, G, nchunk, 128, R)
        .transpose(0, 1, 4, 3, 2, 5)                      # [8, g, 128, c, G, R]
    ).reshape(N_CORES, ngroup, 128, nchunk * KR)
    v_sw = np.ascontiguousarray(
        (alpha * projection_v[sa]).astype(mmdt_np)).reshape(
        N_CORES, ngroup, KR, H)

    in_maps = []
    for core in range(N_CORES):
        m = {
            "u_sw": u_sw[core],
            "v_sw": v_sw[core],
            "hT_sw": hT_sw[core],
            "ident": ident,
        }
        in_maps.append(m)

    nc = _get_program(nslot, H, R, VARIANT)

    from concourse.bass_utils import run_bass_kernel_spmd
    res = run_bass_kernel_spmd(nc, in_maps, list(range(N_CORES)))

    out = np.empty_like(h)
    for core in range(N_CORES):
        o = np.asarray(res.results[core]["out_pk"]).reshape(nrows, H)
        out[row_idx[core][row_valid[core]]] = o[row_valid[core]]
    # gamma/beta are applied host-side (the device computes plain LayerNorm);
    # for the common gamma=1/beta=0 case this is a no-op.
    if use_gamma:
        out *= ln_gamma
    if use_beta:
        out += ln_beta
    return out



# revision 2
# speedup vs baseline: 1.0088x; 1.0088x over previous
"""Trainium2 Bass kernel for DiversityInjection (MoE-style per-agent low-rank
perturbation + LayerNorm) — v2.

Strategy: expert-parallel over the 256 agents. The host assigns agents to the
8 cores (LPT balance: ~32 agents / ~1024 tokens per core), then bin-packs each
core's agents into full 128-token tiles with up to 4 variable-length agent
segments per tile (FFD). All three per-tile operands are fused into one DRAM
tensor so every input DMA moves 768KB:

  in_d[t] = [128, 3072] = [ u_stack (1024) | hT (1024) | v_stack (1024) ]

Per tile on device (T=128 tokens, KR=4*32=128 stacked rank):
  mm1:   psum1[128, 128] = sum_c u_chunk[128,128]^T @ hT_chunk[128,128]
         (full cross-product of 4 agents x 128 tokens)
  mask:  cross-segment garbage is zeroed by blk = psum1 * mask, where
         mask = ecol^T @ erow (one-hot segment matmul, precomputed for all
         tiles at startup into SBUF)
  mm2:   psum2[128, 512] per half = blk^T @ v_half, then residual h added
         via 4 identity-transpose matmuls (bf16 -> fp32 psum accumulate)
  LN:    bn_stats/bn_aggr + scalar activation, fused scale+bias

Outputs are batched 3 tiles per 768KB store. The padded output is scattered
back to original token order on the host.
"""

import os
import sys

for _p in ("/opt/trn_rl_repo", "/root/.axon_site/_ro/trn_rl_repo"):
    if os.path.isdir(_p) and _p not in sys.path:
        sys.path.insert(0, _p)

import numpy as np

N_CORES = 8
T = 128            # tokens per tile
NSEG = 4           # max agent segments per tile
RANK = 32
KR = NSEG * RANK   # 128
MAXNT = 16         # fallback to numpy beyond this many tiles per core
OBATCH = 3         # max tiles per output store
ALPHA_MAX = 5.0
LN_EPS = 1e-5


def _batches(nt):
    """Output store batching: groups of 3 early on, then single-tile stores
    for the last tiles so the end-of-kernel store tail is minimal and
    overlaps the remaining compute."""
    sizes = []
    rem = nt
    while rem > 3:
        sizes.append(OBATCH)
        rem -= OBATCH
    sizes += [1] * rem
    return sizes

_PROGRAM_CACHE = {}


def _reference_numpy(h, log_alpha, ln_gamma, ln_beta, projection_u, projection_v,
                     agent_ids):
    """Fallback pure-numpy implementation (used only if packing does not fit)."""
    num_agents = projection_u.shape[0]
    ids = agent_ids % num_agents
    alpha = min(np.exp(np.float32(log_alpha)), np.float32(ALPHA_MAX))
    out = np.empty_like(h)
    for a in range(num_agents):
        m = ids == a
        if not m.any():
            continue
        hb = h[m]
        pert = (hb @ projection_u[a]) @ projection_v[a]
        out[m] = hb + alpha * pert
    mean = out.mean(-1, keepdims=True, dtype=np.float64)
    var = out.var(-1, keepdims=True, dtype=np.float64)
    out = (out - mean) / np.sqrt(var + LN_EPS)
    return (out * ln_gamma + ln_beta).astype(h.dtype)


def _build_program(nt, hidden):
    """Build the per-core Bass program. Same program runs SPMD on all 8 cores."""
    from contextlib import ExitStack

    import concourse.bacc as bacc
    import concourse.mybir as mybir
    import concourse.tile as tile

    assert hidden == 1024
    nchunk = hidden // 128
    mmdt = mybir.dt.bfloat16
    bsizes = _batches(nt)
    nob = len(bsizes)

    nc = bacc.Bacc("TRN2", target_bir_lowering=False, debug=False)

    in_d = nc.dram_tensor("in_all", (nt, 128, 3 * hidden), mmdt,
                          kind="ExternalInput")
    id_d = nc.dram_tensor("ident", (128, 128), mmdt, kind="ExternalInput")
    ecol_d = nc.dram_tensor("ecol", (NSEG, KR), mmdt, kind="ExternalInput")
    erow_d = nc.dram_tensor("erow", (NSEG, nt * T), mmdt, kind="ExternalInput")
    out_d = nc.dram_tensor("out_pk", (nob, 128, OBATCH * hidden), mmdt,
                           kind="ExternalOutput")

    U0, H0, V0 = 0, hidden, 2 * hidden  # column offsets inside an in-tile

    with tile.TileContext(nc) as tc, ExitStack() as ctx:
        # no buffer reuse anywhere: every input tile and output batch gets its
        # own SBUF slot so all DMAs are issued upfront and stream continuously
        inpool = ctx.enter_context(tc.tile_pool(name="in", bufs=nt))
        bpool = ctx.enter_context(tc.tile_pool(name="blk", bufs=3))
        spool = ctx.enter_context(tc.tile_pool(name="stats", bufs=8))
        opool = ctx.enter_context(tc.tile_pool(name="o", bufs=nob))
        cpool = ctx.enter_context(tc.tile_pool(name="const", bufs=1))
        p1pool = ctx.enter_context(tc.tile_pool(name="psum1", bufs=2, space="PSUM"))
        p2pool = ctx.enter_context(tc.tile_pool(name="psum2", bufs=3, space="PSUM"))

        eps_t = cpool.tile([128, 1], mybir.dt.float32)
        nc.vector.memset(eps_t[:], LN_EPS)
        # consts go FIRST on the same sync ring as the input stream: the ring
        # is FIFO, so they land in <1us instead of trickling behind the big
        # transfers on a contended second ring
        id_t = cpool.tile([128, 128], mmdt)
        nc.sync.dma_start(id_t[:], id_d[:])
        ecol_t = cpool.tile([NSEG, KR], mmdt)
        nc.sync.dma_start(ecol_t[:], ecol_d[:])
        erow_t = cpool.tile([NSEG, nt * T], mmdt)
        nc.sync.dma_start(erow_t[:], erow_d[:])

        dmas = []
        for t in range(nt):
            in_t = inpool.tile([128, 3 * hidden], mmdt)
            nc.sync.dma_start(in_t[:], in_d[t])
            dmas.append(in_t)

        # precompute segment masks for all tiles into SBUF (overlaps DMA fill)
        mask_sb = cpool.tile([128, nt * T], mmdt)
        for b in range(0, nt * T, T):
            pm = p1pool.tile([128, T], mybir.dt.float32, tag="p1")
            nc.tensor.matmul(pm[:], ecol_t[:], erow_t[:, b:b + T],
                             start=True, stop=True)
            nc.vector.tensor_copy(mask_sb[:, b:b + T], pm[:])

        # warm-keeper: dead matmuls that keep the PE array active through the
        # DMA fill window so the HAM clock gate reaches 8/8 (2.4 GHz) before
        # the real tile stream starts, instead of running the whole stream
        # at the cold 1.2 GHz rate
        for i in range(16):
            pm = p1pool.tile([128, T], mybir.dt.float32, tag="p1")
            nc.tensor.matmul(pm[:], ecol_t[:], erow_t[:, 0:T],
                             start=True, stop=True)

        def emit_mm1(t, in_t):
            psum1 = p1pool.tile([128, T], mybir.dt.float32, tag="p1")
            for c in range(nchunk):
                nc.tensor.matmul(
                    psum1[:],
                    in_t[:, U0 + c * 128:U0 + (c + 1) * 128],
                    in_t[:, H0 + c * 128:H0 + (c + 1) * 128],
                    start=(c == 0), stop=(c == nchunk - 1),
                )
            blk = bpool.tile([KR, T], mmdt)
            nc.vector.scalar_tensor_tensor(
                blk[:], psum1[:], 1.0, mask_sb[:, t * T:(t + 1) * T],
                mybir.AluOpType.mult, mybir.AluOpType.mult)
            return blk

        cur = emit_mm1(0, dmas[0])
        o_t = opool.tile([128, OBATCH * hidden], mmdt)
        bi, bj = 0, 0  # output batch index / slot within batch

        for t in range(nt):
            blk, in_t = cur, dmas[t]

            # next tile's mm1 + blk extraction go FIRST so the DVE produces
            # blk(t+1) before it dives into tile t's LN-stats chain — the PE
            # never waits on blk at the next tile boundary
            if t + 1 < nt:
                cur = emit_mm1(t + 1, dmas[t + 1])

            # per 512-wide half: mm2 for the half, then the residual-h
            # transposes for that half (bf16 ins accumulate into fp32 psum),
            # then bn_stats on it — LN stats of half 0 overlap half 1
            psum2 = p2pool.tile([128, hidden], mybir.dt.float32)
            stats = spool.tile([128, 6 * (hidden // 512)], mybir.dt.float32)

            def emit_half(q):
                nc.tensor.matmul(
                    psum2[:, q * 512:(q + 1) * 512],
                    blk[:],
                    in_t[:, V0 + q * 512:V0 + (q + 1) * 512],
                    start=True, stop=True,
                )
                for c in range(4 * q, 4 * (q + 1)):
                    nc.tensor.matmul(
                        psum2[:, c * 128:(c + 1) * 128],
                        in_t[:, H0 + c * 128:H0 + (c + 1) * 128],
                        id_t[:],
                        start=False, stop=True,
                        skip_group_check=True,
                    )
                nc.vector.bn_stats(stats[:, q * 6:(q + 1) * 6],
                                   psum2[:, q * 512:(q + 1) * 512])

            emit_half(0)
            emit_half(1)

            aggr = spool.tile([128, 2], mybir.dt.float32)
            nc.vector.bn_aggr(aggr[:],
                              stats[:].rearrange("p (c s) -> p c s", s=3))
            std = spool.tile([128, 1], mybir.dt.float32)
            nc.scalar.activation(std[:], aggr[:, 1:2],
                                 mybir.ActivationFunctionType.Sqrt,
                                 bias=eps_t[:, 0:1])
            rstd = spool.tile([128, 1], mybir.dt.float32)
            nc.vector.reciprocal(rstd[:], std[:])
            nmr = spool.tile([128, 1], mybir.dt.float32)
            nc.vector.scalar_tensor_tensor(nmr[:], aggr[:, 0:1], -1.0,
                                           rstd[:],
                                           mybir.AluOpType.mult,
                                           mybir.AluOpType.mult)
            nc.scalar.activation(o_t[:, bj * hidden:(bj + 1) * hidden],
                                 psum2[:],
                                 mybir.ActivationFunctionType.Identity,
                                 bias=nmr[:, 0:1], scale=rstd[:, 0:1])
            bj += 1
            if bj == bsizes[bi]:
                w = bsizes[bi] * hidden
                # last store goes on the idle sync HWDGE ring (0.6us
                # first-byte vs ~2us SWDGE) to shrink the end-of-kernel tail
                eng = nc.sync if bi == nob - 1 else nc.gpsimd
                eng.dma_start(out_d[bi][:, 0:w], o_t[:, 0:w])
                bi += 1
                bj = 0
                if t + 1 < nt:
                    o_t = opool.tile([128, OBATCH * hidden], mmdt)

    nc.finalize()
    return nc


def _get_program(nt, hidden):
    key = (nt, hidden)
    if key not in _PROGRAM_CACHE:
        _PROGRAM_CACHE[key] = _build_program(nt, hidden)
    return _PROGRAM_CACHE[key]


def _pack(ids, A):
    """Assign agents to cores (LPT) and bin-pack into (token<=128, seg<=4)
    tiles (FFD). Returns (nt, tiles) where tiles[core] = list of list of
    (agent, start, count) segments indexing the sorted-by-agent token order."""
    counts = np.bincount(ids, minlength=A)
    starts = np.zeros(A + 1, np.int64)
    np.cumsum(counts, out=starts[1:])

    # items: (count, agent, start) — split any oversized agent run
    items = []
    for a in range(A):
        n, s = int(counts[a]), int(starts[a])
        while n > 0:
            take = min(n, T)
            items.append((take, a, s))
            s += take
            n -= take
    items.sort(key=lambda x: -x[0])

    core_tok = np.zeros(N_CORES, np.int64)
    core_items = [[] for _ in range(N_CORES)]
    for it in items:
        c = int(np.argmin(core_tok))
        core_tok[c] += it[0]
        core_items[c].append(it)

    tiles = []
    for c in range(N_CORES):
        bins = []  # (tokens, [(agent, start, count)])
        for n, a, s in core_items[c]:
            for b in bins:
                if b[0] + n <= T and len(b[1]) < NSEG:
                    b[0] += n
                    b[1].append((a, s, n))
                    break
            else:
                bins.append([n, [(a, s, n)]])
        tiles.append([b[1] for b in bins])
    nt = max(len(tb) for tb in tiles)
    return nt, tiles


def kernel(h, log_alpha, ln_gamma, ln_beta, projection_u, projection_v,
           agent_ids):
    h = np.asarray(h, dtype=np.float32)
    projection_u = np.asarray(projection_u, dtype=np.float32)
    projection_v = np.asarray(projection_v, dtype=np.float32)
    ln_gamma = np.asarray(ln_gamma, dtype=np.float32)
    ln_beta = np.asarray(ln_beta, dtype=np.float32)
    ids_raw = np.asarray(agent_ids)
    log_alpha = np.float32(np.asarray(log_alpha))

    B, H = h.shape
    A, H2, R = projection_u.shape
    ids = (ids_raw.astype(np.int64) % A).astype(np.int32)

    if H != 1024 or H2 != H or R != RANK or projection_v.shape != (A, R, H):
        return _reference_numpy(h, log_alpha, ln_gamma, ln_beta, projection_u,
                                projection_v, agent_ids)

    alpha = np.float32(min(np.exp(log_alpha), np.float32(ALPHA_MAX)))
    use_gamma = not np.all(ln_gamma == 1.0)
    use_beta = not np.all(ln_beta == 0.0)

    order = np.argsort(ids, kind="stable").astype(np.int64)
    nt, tiles = _pack(ids, A)
    if nt > MAXNT:
        return _reference_numpy(h, log_alpha, ln_gamma, ln_beta, projection_u,
                                projection_v, agent_ids)

    import ml_dtypes
    bf16 = ml_dtypes.bfloat16
    nchunk = H // 128

    # per (core, tile): agents [NSEG], seg validity, and per-row token/seg maps
    sa = np.zeros((N_CORES, nt, NSEG), np.int64)
    sv = np.zeros((N_CORES, nt, NSEG), bool)
    row_idx = np.zeros((N_CORES, nt, T), np.int64)
    row_valid = np.zeros((N_CORES, nt, T), bool)
    row_seg = np.zeros((N_CORES, nt, T), np.int64)  # segment id per row
    for c in range(N_CORES):
        for ti, segs in enumerate(tiles[c]):
            r = 0
            for si, (a, s, n) in enumerate(segs):
                sa[c, ti, si] = a
                sv[c, ti, si] = True
                row_idx[c, ti, r:r + n] = order[s:s + n]
                row_valid[c, ti, r:r + n] = True
                row_seg[c, ti, r:r + n] = si
                r += n

    # h gathered per row, zeroed on padding: [8, nt, T, H]
    h_pk = np.where(row_valid[..., None], h[row_idx], 0.0).astype(bf16)
    # hT: [8, nt, 128p, c, t]
    hT = np.ascontiguousarray(
        h_pk.reshape(N_CORES, nt, T, nchunk, 128).transpose(0, 1, 4, 3, 2)
    ).reshape(N_CORES, nt, 128, H)

    # u_stack: [8, nt, 128p, c, s, r]
    u_sel = projection_u[sa] * sv[..., None, None]          # [8, nt, 4, H, R]
    u_pk = np.ascontiguousarray(
        u_sel.reshape(N_CORES, nt, NSEG, nchunk, 128, RANK)
        .transpose(0, 1, 4, 3, 2, 5)
    ).reshape(N_CORES, nt, 128, H).astype(bf16)

    # v_stack: [8, nt, s*r, H] with alpha folded in
    v_pk = (projection_v[sa] * (alpha * sv[..., None, None])).reshape(
        N_CORES, nt, KR, H).astype(bf16)

    in_all = np.concatenate([u_pk, hT, v_pk], axis=3)

    # one-hot segment rows: erow[s, t] = 1 iff row t belongs to segment s
    erow = (row_seg[..., None, :] == np.arange(NSEG)[None, None, :, None])
    erow = (erow & row_valid[..., None, :]).astype(bf16)    # [8, nt, 4, T]
    erow = np.ascontiguousarray(erow.transpose(0, 2, 1, 3)).reshape(
        N_CORES, NSEG, nt * T)

    ecol = np.zeros((NSEG, KR), bf16)
    for s in range(NSEG):
        ecol[s, s * RANK:(s + 1) * RANK] = 1
    ident = np.eye(128, dtype=bf16)

    in_maps = []
    for core in range(N_CORES):
        in_maps.append({
            "in_all": in_all[core],
            "ident": ident,
            "ecol": ecol,
            "erow": erow[core],
        })

    nc = _get_program(nt, H)

    from concourse.bass_utils import run_bass_kernel_spmd
    res = run_bass_kernel_spmd(nc, in_maps, list(range(N_CORES)))

    bsizes = _batches(nt)
    out = np.empty_like(h)
    for core in range(N_CORES):
        o = np.asarray(res.results[core]["out_pk"])        # [nob, 128, 3*H]
        o = np.concatenate(
            [o[b, :, :w * H].reshape(128, w, H).transpose(1, 0, 2)
             for b, w in enumerate(bsizes)], axis=0)       # [nt, 128, H]
        rv = row_valid[core].reshape(nt * T)
        out[row_idx[core].reshape(nt * T)[rv]] = o.reshape(nt * T, H)[rv]
    # gamma/beta are applied host-side (the device computes plain LayerNorm);
    # for the common gamma=1/beta=0 case this is a no-op.
    if use_gamma:
        out *= ln_gamma
    if use_beta:
        out += ln_beta
    return out
